# revision 16
# baseline (speedup 1.0000x reference)
"""Trainium2 Bass kernel for a CTRNN forward pass.

Model (per trial b, timestep t):
    ah_{t+1} = 0.9*ah_t + 0.1*(W_h2ah @ h_t + W_x2ah @ x_t + b_x2ah)
    h_{t+1}  = max(tanh(ah_{t+1}), 0) + noise_t
    hstore[t] = h_{t+1}
    y = hstore @ W_h2y.T + b_h2y

Sharding: data-parallel over the 256 trials -> 32 trials per core on 8 cores,
weights replicated.  All state is kept transposed ([neuron, batch]) so the
recurrent matmul output lands directly in the layout the elementwise chain
and the next step's matmul need; per-partition-only parameter ops are avoided
entirely (bias folded into the precomputed input projection).

Device phases (identical SPMD program, per core):
  1. input projection XPT[:, t] = 0.1*(W_x2ah @ x_t + b_x2ah), kept in SBUF
  2. sequential scan over 500 steps (16 fp16 matmuls + 3 fused elementwise
     ops per step), h streamed out to DRAM
  3. output projection y = hstore @ W_h2y.T + b_h2y from the DRAM h history
"""

import numpy as np

B_FULL, T, NI, NR, NO = 256, 500, 128, 512, 128
NCORES = 8
B = B_FULL // NCORES          # 32 trials per core
CH = NR // 128                # 4 partition chunks of the recurrent state
ALPHA = 0.1                   # dt/tau
DECAY = 1.0 - ALPHA
TBLK = 16                     # timesteps per 512-column projection block
NBLK = (T + TBLK - 1) // TBLK
TB = T * B
G = 10                        # steps per DMA group for noise-in / h-out streams
NG = T // G

_CACHE = {}


def _build_module():
    from contextlib import ExitStack

    import concourse.bacc as bacc
    import concourse.tile as tile
    from concourse import mybir

    dt = mybir.dt
    Alu = mybir.AluOpType
    Act = mybir.ActivationFunctionType

    nc = bacc.Bacc(
        "TRN2",
        target_bir_lowering=False,
        debug=False,
        enable_asserts=False,
        num_devices=NCORES,
    )

    xT_d = nc.dram_tensor("xT", [NI, TB], dt.float16, kind="ExternalInput")
    noise_d = nc.dram_tensor("noiseT", [T, 128, CH * B], dt.float32, kind="ExternalInput")
    wrec_d = nc.dram_tensor("wrec", [NR, NR], dt.float16, kind="ExternalInput")
    win_d = nc.dram_tensor("win", [NI, NR], dt.float16, kind="ExternalInput")
    bx_d = nc.dram_tensor("bx", [1, NR], dt.float16, kind="ExternalInput")
    wout_d = nc.dram_tensor("wout", [NR, NO], dt.float16, kind="ExternalInput")
    by_d = nc.dram_tensor("by", [1, NO], dt.float16, kind="ExternalInput")
    ones_d = nc.dram_tensor("ones", [1, 512], dt.float16, kind="ExternalInput")
    ah0_d = nc.dram_tensor("ah0b", [128, CH * B], dt.float32, kind="ExternalInput")
    h0_d = nc.dram_tensor("h0b", [128, CH * B], dt.float16, kind="ExternalInput")
    hst_d = nc.dram_tensor("hstoreT", [T, 128, CH * B], dt.float16, kind="ExternalOutput")
    yT_d = nc.dram_tensor("yT", [NO, TB], dt.float32, kind="ExternalOutput")

    with tile.TileContext(nc) as tc, ExitStack() as ctx:
        const = ctx.enter_context(tc.tile_pool(name="const", bufs=1))
        state = ctx.enter_context(tc.tile_pool(name="state", bufs=1))
        nzp = ctx.enter_context(tc.tile_pool(name="nz", bufs=3))
        tmp = ctx.enter_context(tc.tile_pool(name="tmp", bufs=3))
        drain = ctx.enter_context(tc.tile_pool(name="drain", bufs=3))
        hsto = ctx.enter_context(tc.tile_pool(name="hsto", bufs=2))
        hstp = ctx.enter_context(tc.tile_pool(name="hst", bufs=8))
        ps_scan = ctx.enter_context(tc.tile_pool(name="ps_scan", bufs=2, space="PSUM"))
        ps_proj = ctx.enter_context(tc.tile_pool(name="ps_proj", bufs=2, space="PSUM"))

        # ---- constants / weights into SBUF
        wrec_sb = const.tile([128, CH * NR], dt.float16)  # [k_in_chunk, (ck, n)]
        nc.sync.dma_start(
            wrec_sb[:].rearrange("p (c n) -> p c n", c=CH),
            wrec_d.ap().rearrange("(c p) n -> p c n", p=128),
        )
        win_sb = const.tile([128, NR], dt.float16)
        nc.sync.dma_start(win_sb[:], win_d.ap())
        wout_sb = const.tile([128, CH * NO], dt.float16)  # [n_in_chunk, (cn, o)]
        nc.sync.dma_start(
            wout_sb[:].rearrange("p (c o) -> p c o", c=CH),
            wout_d.ap().rearrange("(c p) o -> p c o", p=128),
        )
        bx_sb = const.tile([1, NR], dt.float16)
        nc.sync.dma_start(bx_sb[:], bx_d.ap())
        by_sb = const.tile([1, NO], dt.float16)
        nc.sync.dma_start(by_sb[:], by_d.ap())
        ones_sb = const.tile([1, 512], dt.float16)
        nc.sync.dma_start(ones_sb[:], ones_d.ap())
        xT_sb = const.tile([128, TB], dt.float16)
        nc.sync.dma_start(xT_sb[:], xT_d.ap())
        xpt_sb = const.tile([128, T * CH * B], dt.float16)  # [p, (t, c, b)]
        xpt_v = xpt_sb[:].rearrange("p (t c b) -> p t c b", t=T, c=CH, b=B)

        ahT = state.tile([128, CH * B], dt.float32)
        nc.sync.dma_start(ahT[:], ah0_d.ap())
        h0_sb = state.tile([128, CH * B], dt.float16)
        nc.sync.dma_start(h0_sb[:], h0_d.ap())

        # ---- phase 1: input projection (+bias), transposed, resident in SBUF
        for j in range(NBLK):
            t0 = j * TBLK
            nt = min(TBLK, T - t0)
            ncol = nt * B
            for cn in range(CH):
                ps = ps_proj.tile([128, 512], dt.float32, tag="psx")
                nc.tensor.matmul(
                    ps[:, :ncol],
                    lhsT=win_sb[:, 128 * cn : 128 * cn + 128],
                    rhs=xT_sb[:, t0 * B : t0 * B + ncol],
                    start=True,
                    stop=False,
                )
                nc.tensor.matmul(
                    ps[:, :ncol],
                    lhsT=bx_sb[:, 128 * cn : 128 * cn + 128],
                    rhs=ones_sb[:, :ncol],
                    start=False,
                    stop=True,
                )
                dst = xpt_v[:, t0 : t0 + nt, cn, :]
                src = ps[:, :ncol].rearrange("p (t b) -> p t b", b=B)
                if (j * CH + cn) % 2 == 0:
                    nc.vector.tensor_copy(dst, src)
                else:
                    nc.scalar.copy(dst, src)

        # ---- phase 2: the scan (DMA streams grouped over G steps)
        W = CH * B  # 128, one step's state width
        h_cur = h0_sb[:]
        for g in range(NG):
            nzt = nzp.tile([128, G * W], dt.float32, tag="nzt")
            nc.gpsimd.dma_start(
                nzt[:].rearrange("p (t e) -> p t e", t=G),
                noise_d.ap()[g * G : (g + 1) * G].rearrange("t p e -> p t e"),
            )
            hstg = hsto.tile([128, G * W], dt.float16, tag="hstg")
            for i in range(G):
                t = g * G + i

                # Y_t = 0.9*ah_t + xpt_t   (off critical path: runs during matmuls)
                Y = tmp.tile([128, W], dt.float32, tag="Y")
                nc.vector.scalar_tensor_tensor(
                    out=Y[:],
                    in0=ahT[:],
                    scalar=DECAY,
                    in1=xpt_v[:, t, :, :],
                    op0=Alu.mult,
                    op1=Alu.add,
                )

                ps = ps_scan.tile([128, W], dt.float32, tag="ps")
                for cn in range(CH):
                    for ck in range(CH):
                        nc.tensor.matmul(
                            ps[:, B * cn : B * cn + B],
                            lhsT=wrec_sb[
                                :, NR * ck + 128 * cn : NR * ck + 128 * cn + 128
                            ],
                            rhs=h_cur[:, B * ck : B * ck + B],
                            start=(ck == 0),
                            stop=(ck == CH - 1),
                        )

                # ah_{t+1} = 0.1*(W@h) + Y
                nc.vector.scalar_tensor_tensor(
                    out=ahT[:],
                    in0=ps[:],
                    scalar=ALPHA,
                    in1=Y[:],
                    op0=Alu.mult,
                    op1=Alu.add,
                )
                Tt = tmp.tile([128, W], dt.float32, tag="Tt")
                nc.scalar.activation(Tt[:], ahT[:], Act.Tanh)
                # h_{t+1} = max(tanh, 0) + noise
                h_next = hstg[:, i * W : (i + 1) * W]
                nc.vector.scalar_tensor_tensor(
                    out=h_next,
                    in0=Tt[:],
                    scalar=0.0,
                    in1=nzt[:, i * W : (i + 1) * W],
                    op0=Alu.max,
                    op1=Alu.add,
                )
                h_cur = h_next
            nc.sync.dma_start(
                hst_d.ap()[g * G : (g + 1) * G].rearrange("t p e -> p t e"),
                hstg[:].rearrange("p (t e) -> p t e", t=G),
            )

        # ---- phase 3: output projection
        for j in range(NBLK):
            t0 = j * TBLK
            nt = min(TBLK, T - t0)
            ncol = nt * B
            psy = ps_proj.tile([128, 512], dt.float32, tag="psy")
            for ck in range(CH):
                hh = hstp.tile([128, TBLK * B], dt.float16, tag="hh")
                nc.sync.dma_start(
                    hh[:, :ncol].rearrange("p (t b) -> p t b", b=B),
                    hst_d.ap()[t0 : t0 + nt, :, ck * B : (ck + 1) * B].rearrange(
                        "t p b -> p t b"
                    ),
                )
                nc.tensor.matmul(
                    psy[:, :ncol],
                    lhsT=wout_sb[:, 128 * ck : 128 * ck + 128],
                    rhs=hh[:, :ncol],
                    start=(ck == 0),
                    stop=False,
                )
            nc.tensor.matmul(
                psy[:, :ncol],
                lhsT=by_sb[:, :],
                rhs=ones_sb[:, :ncol],
                start=False,
                stop=True,
            )
            ysb = drain.tile([128, 512], dt.float32, tag="ysb")
            nc.vector.tensor_copy(ysb[:, :ncol], psy[:, :ncol])
            nc.sync.dma_start(yT_d.ap()[:, t0 * B : t0 * B + ncol], ysb[:, :ncol])

    nc.compile()
    return nc


def _get_module():
    if "nc" not in _CACHE:
        _CACHE["nc"] = _build_module()
    return _CACHE["nc"]


def _prep_inputs(x, activity_noise, W_x2ah, b_x2ah, W_h2ah, W_h2y, b_h2y, ah0):
    f16, f32 = np.float16, np.float32
    wrec = np.ascontiguousarray(W_h2ah.astype(f32).T).astype(f16)            # [k, n]
    win = np.ascontiguousarray((ALPHA * W_x2ah.astype(f32)).T).astype(f16)   # [i, n]
    bx = (ALPHA * b_x2ah.astype(f32)).reshape(1, NR).astype(f16)
    wout = np.ascontiguousarray(W_h2y.astype(f32).T).astype(f16)             # [n, o]
    by = b_h2y.astype(f32).reshape(1, NO).astype(f16)
    ones = np.ones((1, 512), dtype=f16)

    ah0 = ah0.astype(f32)
    h0 = np.maximum(np.tanh(ah0), 0.0).astype(f32)
    # [128, CH*B]: row p, col c*B+b holds neuron n = 128*c + p
    ah0b = np.broadcast_to(
        ah0.reshape(CH, 128).T[:, :, None], (128, CH, B)
    ).reshape(128, CH * B)
    h0b = (
        np.broadcast_to(h0.reshape(CH, 128).T[:, :, None], (128, CH, B))
        .reshape(128, CH * B)
        .astype(f16)
    )

    in_maps = []
    for c in range(NCORES):
        sl = slice(c * B, (c + 1) * B)
        xc = x[sl].astype(f32)                       # [B, T, NI]
        xT = np.ascontiguousarray(xc.transpose(2, 1, 0)).reshape(NI, TB).astype(f16)
        nz = activity_noise[sl].astype(f32)          # [B, T, NR]
        # [T, 128, CH*B]: nzT[t, p, c*B+b] = noise[b, t, 128*c+p]
        nzT = np.ascontiguousarray(
            nz.reshape(B, T, CH, 128).transpose(1, 3, 2, 0)
        ).reshape(T, 128, CH * B)
        in_maps.append(
            {
                "xT": xT,
                "noiseT": nzT,
                "wrec": wrec,
                "win": win,
                "bx": bx,
                "wout": wout,
                "by": by,
                "ones": ones,
                "ah0b": np.ascontiguousarray(ah0b),
                "h0b": np.ascontiguousarray(h0b),
            }
        )
    return in_maps


def kernel(x, activity_noise, W_x2ah, b_x2ah, W_h2ah, W_h2y, b_h2y, ah0):
    from concourse import bass_utils

    nc = _get_module()
    in_maps = _prep_inputs(
        x, activity_noise, W_x2ah, b_x2ah, W_h2ah, W_h2y, b_h2y, ah0
    )
    res = bass_utils.run_bass_kernel_spmd(nc, in_maps, core_ids=list(range(NCORES)))

    y = np.empty((B_FULL, T, NO), dtype=np.float32)
    hstore = np.empty((B_FULL, T, NR), dtype=np.float32)
    for c, out in enumerate(res.results):
        sl = slice(c * B, (c + 1) * B)
        y[sl] = _assemble_y(out["yT"])
        hstore[sl] = _assemble_h(out["hstoreT"])
    return y, hstore


def _assemble_y(yT):
    # yT [NO, T*B] -> [B, T, NO]
    return yT.reshape(NO, T, B).transpose(2, 1, 0)


def _assemble_h(hstT):
    # hstT [T, 128, CH*B] -> [B, T, NR] with n = 128*c + p
    return (
        hstT.astype(np.float32)
        .reshape(T, 128, CH, B)
        .transpose(3, 0, 2, 1)
        .reshape(B, T, NR)
    )
